# revision 2
# baseline (speedup 1.0000x reference)
"""LCALSTM recall step on 8 Trainium2 NeuronCores.

Strategy (row-sharded episodic memory):
- mem [65536, 1024] row-sharded: 8192 rows/core (32MB fp32).
- LSTM gate rows sharded 8-way (each core computes its 128-slice of
  c_t/o_t); tiny AllGather gives every core the full c_t (= query q) and o_t.
- Pass 1 per core: stream local mem tiles [128,1024]; DVE mult+reduce gives
  dot(mem_i, q); ACT square+accum gives ||mem_i||^2. Normalize -> local sims.
- AllGather sims (32KB/core); the 10-cycle LCA competition runs REPLICATED on
  every core (elementwise + a global sum each cycle, done on-chip via
  ACT relu+accum and a PE all-ones broadcast matmul) -- no per-cycle
  collectives. A shadow LCA state over only the LOCAL 8192 slots is updated
  with the same global bias so pass 2 can index local rows at fixed offsets
  (SPMD-safe).
- Pass 2: stream local mem tiles again (first NCACHE tiles are kept in SBUF
  from pass 1), accumulate m_t partial = sum_r wts_r * mem_r via PE matmuls;
  AllReduce (4KB) -> full m_t.
- Heads (dec_act2, actor, critic) computed replicated with a full Wih copy.
- mem_new (input mem with row write_ptr := cm_t) is assembled on host --
  only one 4KB row changes; shipping 256MB through the device would be waste.
"""
import numpy as np


def _ensure_paths():
    import sys
    try:
        import concourse.bass  # noqa: F401
        return
    except ImportError:
        pass
    for p in ("/opt/trn_rl_repo", "/root/.axon_site/_ro/trn_rl_repo"):
        if p not in sys.path:
            sys.path.insert(0, p)
    import concourse.bass  # noqa: F401


H = 1024
DICT = 65536
NC = 8
RL = DICT // NC          # 8192 rows per core
G = RL // 128            # 64 local row-groups
NJ = H // 128            # 8 column groups
CMPT = 0.8
DT = 0.6
EPS = 1e-8
NCYC = 10
NCACHE = 16              # mem tiles kept in SBUF between pass 1 and pass 2
XH = 2176                # padded gate-matmul contraction dim (17*128)
NT_XH = XH // 128        # 17


def _split_multi_waits(nc, mybir):
    """This container's walrus build supports at most ONE semaphore wait (and
    update) per instruction; Tile attaches several. Peel extras onto
    same-engine NoOps."""
    ctr = 0
    for fn in nc.m.functions:
        for bb in fn.blocks:
            il = bb.instructions
            i = 0
            while i < len(il):
                inst = il[i]
                si = inst.sync_info
                if si is None:
                    i += 1
                    continue
                waits = list(si.on_wait) if si.on_wait else []
                upds = list(si.on_update) if si.on_update else []
                changed = False
                if len(waits) > 1:
                    for w in waits[:-1]:
                        nop = mybir.InstNoOp(name=f"wsplit-{ctr}", ins=[], outs=[])
                        ctr += 1
                        nop.engine = inst.engine
                        nop.sync_info = mybir.SyncInfo(on_wait=[w], on_update=[])
                        il.insert(i, nop)
                        i += 1
                    waits = waits[-1:]
                    changed = True
                if len(upds) > 1:
                    for u in upds[1:]:
                        nop = mybir.InstNoOp(name=f"usplit-{ctr}", ins=[], outs=[])
                        ctr += 1
                        nop.engine = inst.engine
                        nop.sync_info = mybir.SyncInfo(on_wait=[], on_update=[u])
                        il.insert(i + 1, nop)
                    upds = upds[:1]
                    changed = True
                if changed:
                    inst.sync_info = mybir.SyncInfo(on_wait=waits, on_update=upds)
                i += 1


def _build_nc():
    import concourse.bass as bass
    import concourse.mybir as mybir
    import concourse.tile as tile

    F32 = mybir.dt.float32
    ALU = mybir.AluOpType
    AF = mybir.ActivationFunctionType
    AX = mybir.AxisListType

    nc = bass.Bass("TRN2", target_bir_lowering=False, debug=False, num_devices=NC)

    # ---- I/O ----
    mem_in = nc.dram_tensor("mem", [RL, H], F32, kind="ExternalInput")
    wcat_in = nc.dram_tensor("wcatT", [XH, 512], F32, kind="ExternalInput")
    xh_in = nc.dram_tensor("xh", [NT_XH, 128], F32, kind="ExternalInput")
    cps_in = nc.dram_tensor("cps", [1, 128], F32, kind="ExternalInput")
    wih_in = nc.dram_tensor("wihr", [H, H], F32, kind="ExternalInput")
    bihc_in = nc.dram_tensor("bihc", [128, NJ], F32, kind="ExternalInput")
    whc_in = nc.dram_tensor("whc", [128, NJ], F32, kind="ExternalInput")
    whd_in = nc.dram_tensor("whd", [128, NJ], F32, kind="ExternalInput")
    bhb_in = nc.dram_tensor("bhb", [128, 1], F32, kind="ExternalInput")
    w5_in = nc.dram_tensor("w5c", [128, 40], F32, kind="ExternalInput")
    b5_in = nc.dram_tensor("b5", [1, 5], F32, kind="ExternalInput")
    id_in = nc.dram_tensor("ident", [128, 128], F32, kind="ExternalInput")

    o_pi = nc.dram_tensor("o_pi", [4], F32, kind="ExternalOutput")
    o_val = nc.dram_tensor("o_val", [1], F32, kind="ExternalOutput")
    o_h2 = nc.dram_tensor("o_h2", [H], F32, kind="ExternalOutput")
    o_cm = nc.dram_tensor("o_cm", [H], F32, kind="ExternalOutput")
    o_m = nc.dram_tensor("o_m", [H], F32, kind="ExternalOutput")

    # ---- internal DRAM ----
    ag1_in = nc.dram_tensor("ag1_in", [2, 128], F32)
    ag1_out = nc.dram_tensor("ag1_out", [NC, 2, 128], F32, addr_space="Shared")
    ag3_in = nc.dram_tensor("ag3_in", [RL], F32)
    ag3_out = nc.dram_tensor("ag3_out", [NC, RL], F32, addr_space="Shared")
    ar4_in = nc.dram_tensor("ar4_in", [H], F32)
    ar4_out = nc.dram_tensor("ar4_out", [H], F32, addr_space="Shared")
    h_dram = nc.dram_tensor("h_dram", [H], F32)
    h2_dram = nc.dram_tensor("h2_dram", [H], F32)

    rg = [list(range(NC))]

    with tile.TileContext(nc) as tc:
        with tc.tile_pool(name="const", bufs=1) as pc, \
             tc.tile_pool(name="wcat", bufs=3) as pw, \
             tc.tile_pool(name="wih", bufs=1) as pwih, \
             tc.tile_pool(name="cache", bufs=1) as pcache, \
             tc.tile_pool(name="mem", bufs=4) as pmem, \
             tc.tile_pool(name="scr", bufs=2) as pscr, \
             tc.tile_pool(name="small", bufs=2) as psm, \
             tc.tile_pool(name="ps", bufs=2, space="PSUM") as pps, \
             tc.tile_pool(name="psacc", bufs=1, space="PSUM") as psacc:

            # ---- constants ----
            ident = pc.tile([128, 128], F32, tag="ident")
            nc.sync.dma_start(ident[:], id_in.ap())
            ones = pc.tile([128, 128], F32, tag="ones")
            nc.gpsimd.memset(ones[:], 1.0)
            negc = pc.tile([128, 128], F32, tag="negc")
            nc.gpsimd.memset(negc[:], -DT * CMPT)

            # ---- persistent Wih rows (also used for dec_act2 at the end) ----
            wih_t = []
            for j in range(NJ):
                wt = pwih.tile([128, H], F32, tag=f"wih{j}")
                nc.sync.dma_start(wt[:], wih_in.ap()[j * 128:(j + 1) * 128, :])
                wih_t.append(wt)

            # ---- LSTM gate slice ----
            xhn = psm.tile([NT_XH, 128], F32, tag="xhn")
            nc.sync.dma_start(xhn[:], xh_in.ap())
            ps_t = pps.tile([128, 128], F32, tag="tsp")
            nc.tensor.matmul(ps_t[:NT_XH and 128, :NT_XH], xhn[:], ident[:NT_XH, :NT_XH],
                             is_transpose=True, start=True, stop=True)
            xhc = psm.tile([128, NT_XH], F32, tag="xhc")
            nc.vector.tensor_copy(xhc[:], ps_t[:, :NT_XH])

            ps_acc = psacc.tile([1, 1024], F32, tag="acc")
            for t in range(NT_XH):
                wct = pw.tile([128, 512], F32, tag="wct")
                nc.sync.dma_start(wct[:], wcat_in.ap()[t * 128:(t + 1) * 128, :])
                nc.tensor.matmul(ps_acc[:, 0:512], xhc[:, t:t + 1], wct[:],
                                 start=(t == 0), stop=(t == NT_XH - 1))
            sig = psm.tile([1, 384], F32, tag="sig")
            nc.scalar.activation(sig[:], ps_acc[:, 0:384], AF.Sigmoid)
            cn = psm.tile([1, 128], F32, tag="cn")
            nc.scalar.activation(cn[:], ps_acc[:, 384:512], AF.Tanh)
            cps = psm.tile([1, 128], F32, tag="cps")
            nc.sync.dma_start(cps[:], cps_in.ap())
            co = psm.tile([1, 256], F32, tag="co")
            nc.vector.tensor_tensor(co[:, 0:128], cps[:], sig[:, 0:128], ALU.mult)
            icn = psm.tile([1, 128], F32, tag="icn")
            nc.vector.tensor_tensor(icn[:], sig[:, 256:384], cn[:], ALU.mult)
            nc.vector.tensor_tensor(co[:, 0:128], co[:, 0:128], icn[:], ALU.add)
            nc.vector.tensor_copy(co[:, 128:256], sig[:, 128:256])
            nc.sync.dma_start(ag1_in.ap().rearrange("a b -> (a b)"), co[:, :])

            nc.gpsimd.collective_compute(
                "AllGather", ALU.bypass, replica_groups=rg,
                ins=[ag1_in.ap()], outs=[ag1_out.ap()])

            # full c_t / o_t as [128, 8] column tiles
            cg_c = psm.tile([NC, 128], F32, tag="cg_c")
            nc.sync.dma_start(cg_c[:], ag1_out.ap()[:, 0, :])
            cg_o = psm.tile([NC, 128], F32, tag="cg_o")
            nc.sync.dma_start(cg_o[:], ag1_out.ap()[:, 1, :])
            ps_c = pps.tile([128, 128], F32, tag="tsp")
            nc.tensor.matmul(ps_c[:, :NC], cg_c[:], ident[:NC, :NC],
                             is_transpose=True, start=True, stop=True)
            c_cols = pc.tile([128, NJ], F32, tag="c_cols")
            nc.vector.tensor_copy(c_cols[:], ps_c[:, :NC])
            ps_o = pps.tile([128, 128], F32, tag="tsp")
            nc.tensor.matmul(ps_o[:, :NC], cg_o[:], ident[:NC, :NC],
                             is_transpose=True, start=True, stop=True)
            o_cols = pc.tile([128, NJ], F32, tag="o_cols")
            nc.vector.tensor_copy(o_cols[:], ps_o[:, :NC])

            # q broadcast [128, 1024] straight from the gathered buffer
            q_b = pc.tile([128, H], F32, tag="q_b")
            nc.sync.dma_start(
                q_b[:].rearrange("p (k j) -> p k j", k=NC),
                ag1_out.ap()[:, 0, :].partition_broadcast(128))

            # ||q||
            scr8 = psm.tile([128, NJ], F32, tag="scr8")
            nc.vector.tensor_tensor(scr8[:], c_cols[:], c_cols[:], ALU.mult)
            qn2 = psm.tile([128, 1], F32, tag="qn2")
            nc.vector.tensor_reduce(qn2[:], scr8[:], AX.X, ALU.add)
            ps_q = pps.tile([128, 1], F32, tag="vec1")
            nc.tensor.matmul(ps_q[:], ones[:], qn2[:], start=True, stop=True)
            qn_b = pc.tile([128, 1], F32, tag="qn_b")
            nc.scalar.activation(qn_b[:], ps_q[:], AF.Sqrt)

            # h_t = o * tanh(c); broadcast via DRAM round-trip
            tanh_c = psm.tile([128, NJ], F32, tag="tanh_c")
            nc.scalar.activation(tanh_c[:], c_cols[:], AF.Tanh)
            h_cols = psm.tile([128, NJ], F32, tag="h_cols")
            nc.vector.tensor_tensor(h_cols[:], o_cols[:], tanh_c[:], ALU.mult)
            ps_hr = pps.tile([128, 128], F32, tag="tsp")
            nc.tensor.matmul(ps_hr[:NJ, :], h_cols[:], ident[:, :],
                             is_transpose=True, start=True, stop=True)
            h_rows = psm.tile([NJ, 128], F32, tag="h_rows")
            nc.vector.tensor_copy(h_rows[:], ps_hr[:NJ, :])
            nc.sync.dma_start(h_dram.ap(), h_rows[:])
            h_b = pc.tile([128, H], F32, tag="h_b")
            nc.sync.dma_start(h_b[:], h_dram.ap().partition_broadcast(128))

            # dec_act (for phi)
            da = psm.tile([128, NJ], F32, tag="da")
            for j in range(NJ):
                prod = pscr.tile([128, H], F32, tag="prod")
                nc.vector.tensor_tensor(prod[:], wih_t[j][:], h_b[:], ALU.mult)
                nc.vector.tensor_reduce(da[:, j:j + 1], prod[:], AX.X, ALU.add)
            bihc = psm.tile([128, NJ], F32, tag="bihc")
            nc.sync.dma_start(bihc[:], bihc_in.ap())
            dec_cols = psm.tile([128, NJ], F32, tag="dec_cols")
            nc.vector.tensor_tensor(dec_cols[:], da[:], bihc[:], ALU.add)
            nc.vector.tensor_scalar(dec_cols[:], dec_cols[:], 0.0, None, ALU.max)

            # phi[0] -> input strength
            whc = psm.tile([128, NJ], F32, tag="whc")
            nc.sync.dma_start(whc[:], whc_in.ap())
            whd = psm.tile([128, NJ], F32, tag="whd")
            nc.sync.dma_start(whd[:], whd_in.ap())
            sc1 = psm.tile([128, NJ], F32, tag="sc1")
            nc.vector.tensor_tensor(sc1[:], c_cols[:], whc[:], ALU.mult)
            ph1 = psm.tile([128, 1], F32, tag="ph1")
            nc.vector.tensor_reduce(ph1[:], sc1[:], AX.X, ALU.add)
            sc2 = psm.tile([128, NJ], F32, tag="sc2")
            nc.vector.tensor_tensor(sc2[:], dec_cols[:], whd[:], ALU.mult)
            ph2 = psm.tile([128, 1], F32, tag="ph2")
            nc.vector.tensor_reduce(ph2[:], sc2[:], AX.X, ALU.add)
            nc.vector.tensor_tensor(ph1[:], ph1[:], ph2[:], ALU.add)
            ps_phi = pps.tile([128, 1], F32, tag="vec1")
            nc.tensor.matmul(ps_phi[:], ones[:], ph1[:], start=True, stop=True)
            bhb = psm.tile([128, 1], F32, tag="bhb")
            nc.sync.dma_start(bhb[:], bhb_in.ap())
            inps_b = pc.tile([128, 1], F32, tag="inps_b")
            nc.scalar.activation(inps_b[:], ps_phi[:], AF.Sigmoid, bias=bhb[:])

            # ---- pass 1: dots + norms over local mem ----
            dot_sb = pc.tile([128, G], F32, tag="dot_sb")
            n2_sb = pc.tile([128, G], F32, tag="n2_sb")
            cache_tiles = []
            for t in range(G):
                if t < NCACHE:
                    mt = pcache.tile([128, H], F32, tag=f"cch{t}")
                    cache_tiles.append(mt)
                else:
                    mt = pmem.tile([128, H], F32, tag="mt")
                nc.sync.dma_start(mt[:], mem_in.ap()[t * 128:(t + 1) * 128, :])
                prod = pscr.tile([128, H], F32, tag="prod")
                nc.vector.tensor_tensor(prod[:], mt[:], q_b[:], ALU.mult)
                nc.vector.tensor_reduce(dot_sb[:, t:t + 1], prod[:], AX.X, ALU.add)
                sq = pscr.tile([128, H], F32, tag="sq")
                nc.scalar.activation(sq[:], mt[:], AF.Square,
                                     accum_out=n2_sb[:, t:t + 1])

            sqn = psm.tile([128, G], F32, tag="sqn")
            nc.scalar.activation(sqn[:], n2_sb[:], AF.Sqrt)
            den = psm.tile([128, G], F32, tag="den")
            nc.vector.tensor_scalar(den[:], sqn[:], qn_b[:], EPS, ALU.mult, ALU.add)
            rden = psm.tile([128, G], F32, tag="rden")
            nc.vector.reciprocal(rden[:], den[:])
            sims_sb = pc.tile([128, G], F32, tag="sims_sb")
            nc.vector.tensor_tensor(sims_sb[:], dot_sb[:], rden[:], ALU.mult)
            nc.sync.dma_start(ag3_in.ap(), sims_sb[:])

            nc.gpsimd.collective_compute(
                "AllGather", ALU.bypass, replica_groups=rg,
                ins=[ag3_in.ap()], outs=[ag3_out.ap()])

            vfull_s = pc.tile([128, 512], F32, tag="vfull_s")
            nc.sync.dma_start(
                vfull_s[:].rearrange("p (k g) -> p k g", k=NC),
                ag3_out.ap().rearrange("k (p g) -> p k g", p=128))

            # ---- LCA (replicated global + local shadow) ----
            inp3 = pc.tile([128, 512], F32, tag="inp3")
            nc.vector.tensor_scalar(inp3[:], vfull_s[:], inps_b[:], DT / (1.0 + DT * CMPT),
                                    ALU.mult, ALU.mult)
            inp3L = pc.tile([128, G], F32, tag="inp3L")
            nc.vector.tensor_scalar(inp3L[:], sims_sb[:], inps_b[:], DT / (1.0 + DT * CMPT),
                                    ALU.mult, ALU.mult)
            a_lca = 1.0 + DT * CMPT
            v = pc.tile([128, 512], F32, tag="v")
            s_p = pc.tile([128, 1], F32, tag="s_p")
            nc.scalar.activation(v[:], inp3[:], AF.Relu, scale=a_lca, accum_out=s_p[:])
            vL = pc.tile([128, G], F32, tag="vL")
            nc.scalar.activation(vL[:], inp3L[:], AF.Relu, scale=a_lca)
            for it in range(1, NCYC):
                ps_b = pps.tile([128, 1], F32, tag="vec1")
                nc.tensor.matmul(ps_b[:], negc[:], s_p[:], start=True, stop=True)
                bias_sb = psm.tile([128, 1], F32, tag="lbias")
                nc.vector.tensor_copy(bias_sb[:], ps_b[:])
                w = pscr.tile([128, 512], F32, tag="lw")
                nc.vector.tensor_tensor(w[:], v[:], inp3[:], ALU.add)
                wL = psm.tile([128, G], F32, tag="lwL")
                nc.vector.tensor_tensor(wL[:], vL[:], inp3L[:], ALU.add)
                nc.scalar.activation(v[:], w[:], AF.Relu, scale=a_lca,
                                     bias=bias_sb[:], accum_out=s_p[:])
                nc.scalar.activation(vL[:], wL[:], AF.Relu, scale=a_lca,
                                     bias=bias_sb[:])

            # ---- pass 2: readout m_t partial ----
            for g in range(G):
                if g < NCACHE:
                    mt = cache_tiles[g]
                else:
                    mt = pmem.tile([128, H], F32, tag="mt2")
                    nc.sync.dma_start(mt[:], mem_in.ap()[g * 128:(g + 1) * 128, :])
                nc.tensor.matmul(ps_acc[:, 0:512], vL[:, g:g + 1], mt[:, 0:512],
                                 start=(g == 0), stop=(g == G - 1),
                                 skip_group_check=True)
                nc.tensor.matmul(ps_acc[:, 512:1024], vL[:, g:g + 1], mt[:, 512:1024],
                                 start=(g == 0), stop=(g == G - 1),
                                 skip_group_check=True)
            m_sb = psm.tile([1, 1024], F32, tag="m_sb")
            nc.scalar.copy(m_sb[:], ps_acc[:])
            nc.sync.dma_start(ar4_in.ap(), m_sb[:])

            nc.gpsimd.collective_compute(
                "AllReduce", ALU.add, replica_groups=rg,
                ins=[ar4_in.ap()], outs=[ar4_out.ap()])

            m8 = psm.tile([NJ, 128], F32, tag="m8")
            nc.sync.dma_start(m8[:], ar4_out.ap().rearrange("(j p) -> j p", j=NJ))
            nc.sync.dma_start(o_m.ap(), m8[:])
            ps_m = pps.tile([128, 128], F32, tag="tsp")
            nc.tensor.matmul(ps_m[:, :NJ], m8[:], ident[:NJ, :NJ],
                             is_transpose=True, start=True, stop=True)
            m_cols = psm.tile([128, NJ], F32, tag="m_cols")
            nc.vector.tensor_copy(m_cols[:], ps_m[:, :NJ])

            # ---- finals ----
            cm_cols = psm.tile([128, NJ], F32, tag="cm_cols")
            nc.vector.tensor_tensor(cm_cols[:], c_cols[:], m_cols[:], ALU.add)
            tanh_cm = psm.tile([128, NJ], F32, tag="tanh_cm")
            nc.scalar.activation(tanh_cm[:], cm_cols[:], AF.Tanh)
            h2_cols = psm.tile([128, NJ], F32, tag="h2_cols")
            nc.vector.tensor_tensor(h2_cols[:], o_cols[:], tanh_cm[:], ALU.mult)

            ps_cmr = pps.tile([128, 128], F32, tag="tsp")
            nc.tensor.matmul(ps_cmr[:NJ, :], cm_cols[:], ident[:, :],
                             is_transpose=True, start=True, stop=True)
            cm_rows = psm.tile([NJ, 128], F32, tag="cm_rows")
            nc.vector.tensor_copy(cm_rows[:], ps_cmr[:NJ, :])
            nc.sync.dma_start(o_cm.ap(), cm_rows[:])

            ps_h2r = pps.tile([128, 128], F32, tag="tsp")
            nc.tensor.matmul(ps_h2r[:NJ, :], h2_cols[:], ident[:, :],
                             is_transpose=True, start=True, stop=True)
            h2_rows = psm.tile([NJ, 128], F32, tag="h2_rows")
            nc.vector.tensor_copy(h2_rows[:], ps_h2r[:NJ, :])
            nc.sync.dma_start(o_h2.ap(), h2_rows[:])
            nc.sync.dma_start(h2_dram.ap(), h2_rows[:])
            h2_b = pc.tile([128, H], F32, tag="h2_b")
            nc.sync.dma_start(h2_b[:], h2_dram.ap().partition_broadcast(128))

            da2 = psm.tile([128, NJ], F32, tag="da2")
            for j in range(NJ):
                prod = pscr.tile([128, H], F32, tag="prod")
                nc.vector.tensor_tensor(prod[:], wih_t[j][:], h2_b[:], ALU.mult)
                nc.vector.tensor_reduce(da2[:, j:j + 1], prod[:], AX.X, ALU.add)
            dec2 = psm.tile([128, NJ], F32, tag="dec2")
            nc.vector.tensor_tensor(dec2[:], da2[:], bihc[:], ALU.add)
            nc.vector.tensor_scalar(dec2[:], dec2[:], 0.0, None, ALU.max)

            w5c = psm.tile([128, 40], F32, tag="w5c")
            nc.sync.dma_start(w5c[:], w5_in.ap())
            hp = psm.tile([128, 5], F32, tag="hp")
            for r in range(5):
                s5 = psm.tile([128, NJ], F32, tag="s5")
                nc.vector.tensor_tensor(s5[:], dec2[:], w5c[:, r * 8:(r + 1) * 8],
                                        ALU.mult)
                nc.vector.tensor_reduce(hp[:, r:r + 1], s5[:], AX.X, ALU.add)
            ps_h5 = pps.tile([1, 5], F32, tag="h5p")
            nc.tensor.matmul(ps_h5[:], ones[:, 0:1], hp[:], start=True, stop=True)
            b5 = psm.tile([1, 5], F32, tag="b5")
            nc.sync.dma_start(b5[:], b5_in.ap())
            h5 = psm.tile([1, 5], F32, tag="h5")
            nc.vector.tensor_tensor(h5[:], ps_h5[:], b5[:], ALU.add)
            ex = psm.tile([1, 4], F32, tag="ex")
            se = psm.tile([1, 1], F32, tag="se")
            nc.scalar.activation(ex[:], h5[:, 0:4], AF.Exp, accum_out=se[:])
            rse = psm.tile([1, 1], F32, tag="rse")
            nc.vector.reciprocal(rse[:], se[:])
            pi = psm.tile([1, 4], F32, tag="pi")
            nc.vector.tensor_scalar(pi[:], ex[:], rse[:], None, ALU.mult)
            nc.sync.dma_start(o_pi.ap(), pi[:])
            nc.sync.dma_start(o_val.ap(), h5[:, 4:5])

    import concourse.mybir as mybir2
    _split_multi_waits(nc, mybir2)
    return nc


_NC_CACHE = {}


def _prep_inputs(inputs):
    f32 = np.float32
    x_t = np.asarray(inputs["x_t"], f32).reshape(-1)        # [1025]
    h_prev = np.asarray(inputs["h_prev"], f32).reshape(-1)  # [1024]
    c_prev = np.asarray(inputs["c_prev"], f32).reshape(-1)
    mem = np.asarray(inputs["mem"], f32)
    Wi2h = np.asarray(inputs["Wi2h"], f32)
    bi2h = np.asarray(inputs["bi2h"], f32)
    Wh2h = np.asarray(inputs["Wh2h"], f32)
    bh2h = np.asarray(inputs["bh2h"], f32)
    Wih = np.asarray(inputs["Wih"], f32)
    bih = np.asarray(inputs["bih"], f32)
    Wactor = np.asarray(inputs["Wactor"], f32)
    bactor = np.asarray(inputs["bactor"], f32)
    Wcritic = np.asarray(inputs["Wcritic"], f32)
    bcritic = np.asarray(inputs["bcritic"], f32)
    Whpc = np.asarray(inputs["Whpc"], f32)
    bhpc = np.asarray(inputs["bhpc"], f32)

    xh = np.zeros(XH, f32)
    xh[:1025] = x_t
    xh[1025] = 1.0
    xh[1026:1026 + H] = h_prev
    xh_n = np.ascontiguousarray(xh.reshape(NT_XH, 128))

    bihc = np.ascontiguousarray(bih.reshape(NJ, 128).T)
    whc = np.ascontiguousarray(Whpc[0, :H].reshape(NJ, 128).T)
    whd = np.ascontiguousarray(Whpc[0, H:].reshape(NJ, 128).T)
    bhb = np.full((128, 1), bhpc[0], f32)
    w5 = np.concatenate([Wactor, Wcritic], axis=0)          # [5, 1024]
    w5c = np.ascontiguousarray(
        w5.reshape(5, NJ, 128).transpose(2, 0, 1).reshape(128, 40))
    b5 = np.concatenate([bactor, bcritic])[None, :].astype(f32)
    ident = np.eye(128, dtype=f32)
    wih_c = np.ascontiguousarray(Wih)

    in_maps = []
    for k in range(NC):
        j_idx = np.arange(128) + k * 128
        row_ids = np.concatenate(
            [j_idx, H + j_idx, 2 * H + j_idx, 3 * H + 2 + j_idx])
        W_cat = np.zeros((512, XH), f32)
        W_cat[:, :1025] = Wi2h[row_ids]
        W_cat[:, 1025] = bi2h[row_ids] + bh2h[row_ids]
        W_cat[:, 1026:1026 + H] = Wh2h[row_ids]
        wcatT = np.ascontiguousarray(W_cat.T)               # [2176, 512]
        in_maps.append({
            "mem": np.ascontiguousarray(mem[k * RL:(k + 1) * RL]),
            "wcatT": wcatT,
            "xh": xh_n,
            "cps": np.ascontiguousarray(c_prev[k * 128:(k + 1) * 128])[None, :],
            "wihr": wih_c,
            "bihc": bihc,
            "whc": whc,
            "whd": whd,
            "bhb": bhb,
            "w5c": w5c,
            "b5": b5,
            "ident": ident,
        })
    return in_maps, mem


def _run(inputs, trace=False, trace_kwargs=None):
    _ensure_paths()
    from concourse.bass_utils import run_bass_kernel_spmd

    if "nc" not in _NC_CACHE:
        _NC_CACHE["nc"] = _build_nc()
    nc = _NC_CACHE["nc"]

    in_maps, mem = _prep_inputs(inputs)
    res = run_bass_kernel_spmd(nc, in_maps, core_ids=list(range(NC)),
                               trace=trace, **(trace_kwargs or {}))
    r0 = res.results[0]

    pi_a_t = r0["o_pi"]
    value_t = r0["o_val"].reshape(1, 1)
    h_t2 = r0["o_h2"].reshape(1, 1, H)
    cm_t = r0["o_cm"].reshape(1, 1, H)
    m_t = r0["o_m"].reshape(1, H)
    mem_new = mem.copy()
    mem_new[int(inputs["write_ptr"])] = r0["o_cm"]
    return (pi_a_t, value_t, h_t2, cm_t, m_t, mem_new), res


def kernel(**inputs):
    outs, _ = _run(inputs, trace=False)
    return outs
